# revision 21
# baseline (speedup 1.0000x reference)
"""Cross-attention kernel for Trainium2 (8 NeuronCores, data-parallel over batch).

Reference computation (per batch b):
    q = (x @ Wq.T) * gamma_q ; k = (ctx @ Wk.T) * gamma_k ; v = (ctx @ Wv.T) * gamma_v
    per head: o = softmax(q k^T / sqrt(dh)) v
    out = (concat_heads(o) @ Wo.T + bo) * gamma_out

Device strategy (per core, 4 batches, n = 4*4096 = 16384 query rows):
  - Everything runs in "transposed world": activations live as [channel | n]
    so the contraction dim is always on partitions.
  - Host folds gamma_q/gamma_k/gamma_v/gamma_out and the 1/sqrt(dh) scale into
    the weights, pre-transposes x and context, and transposes the output back.
  - Heads are packed in PAIRS at partition bases {0, 64} (matmul operand base
    partitions must be 32-aligned and equal for lhsT/rhs).  Score matmuls of a
    pair are row-tiled (40-row K at array rows 0-63 / 64-127, concurrent);
    AV and Z matmuls are col-tiled (64-row M at psum rows 0-63 / 64-127,
    concurrent), landing directly in the [128|512] pair layout the output
    projection wants.
  - Softmax denominator: rb = ones[77|64]^T @ ex is a matmul whose every
    output row is Z = sum_k exp(s) for that column's query -- reduction and
    partition-broadcast in one PE op.  1/Z = exp(-ln Z) on the scalar engine
    (ln and exp live in the same activation table set), then st = ot * (1/Z)
    on the DVE straight from the AV psum.
  - V carries a leading ones-column, so ot row 0 (and 64) is Z; st row 0 is
    Z * (1/Z) = 1 and the output bias rides in wo[pair0, row0, :].
  - Software pipeline over chunks: round ci emits q+scores of chunk ci, the
    attention tail (AV/Z/normalize) of chunk ci-1, and the output projection
    of chunk ci-2, interleaved so the ACT exp chain and the DVE normalize
    chain never stall the PE.
"""

import os
import sys

import ml_dtypes
import numpy as np

BF16NP = ml_dtypes.bfloat16

for _p in ("/opt/trn_rl_repo",):
    if _p not in sys.path and os.path.isdir(_p):
        sys.path.append(_p)

import concourse.bass as bass
import concourse.mybir as mybir
import concourse.tile as tile
from concourse.bass import AP
from concourse.bass_utils import run_bass_kernel_spmd

HEADS = 8
DH = 40
QD = 320            # query/input channel dim == inner dim
CD = 768            # context channel dim
B, NQ, NK = 32, 4096, 77
NCORES = 8
BL = B // NCORES    # batches per core = 4
NLOC = BL * NQ      # query rows per core = 16384
NKL = BL * NK       # context rows per core = 308
CHUNK = 512
NCHUNKS = NLOC // CHUNK          # 32
CHUNKS_PER_BATCH = NQ // CHUNK   # 8
NPAIR = HEADS // 2               # 4 head pairs; pair p = heads (2p, 2p+1)

F32 = mybir.dt.float32
BF16 = mybir.dt.bfloat16

# K-chunking of the contraction dims
DK_Q = [(0, 128), (128, 128), (256, 64)]                       # QD = 320
DK_C = [(i * 128, 128) for i in range(6)]                      # CD = 768
JT = [(0, 128), (128, 128), (256, 64)]                         # out channels 320

LAST_EXEC_NS = None
LAST_RESULTS = None


def _split_multi_waits(nc):
    """Walrus codegen allows at most ONE semaphore wait per instruction.
    Split any instruction with N>1 waits into (N-1) same-engine NoOps, each
    carrying one wait, followed by the original instruction with the last
    wait. Engines execute their streams in order, so this is equivalent."""
    k = 0
    for blk in nc.m.functions[0].blocks:
        insts = list(blk.instructions)
        out = []
        for ins in insts:
            si = getattr(ins, "sync_info", None)
            if si is not None and len(si.on_wait) > 1:
                waits = list(si.on_wait)
                for w in waits[:-1]:
                    nop = mybir.InstNoOp(name=f"wsplit-{k}")
                    k += 1
                    nop.engine = ins.engine
                    nop.sync_info = mybir.SyncInfo(on_wait=[w], on_update=[])
                    out.append(nop)
                ins.sync_info = mybir.SyncInfo(
                    on_wait=[waits[-1]], on_update=list(si.on_update)
                )
            out.append(ins)
        if len(out) != len(insts):
            blk.instructions = out
    return nc


def _build_program():
    nc = bass.Bass(trn_type="TRN2")

    xT = nc.declare_dram_parameter("xT", [QD, NLOC], BF16, isOutput=False)
    cT = nc.declare_dram_parameter("cT", [CD, NKL], BF16, isOutput=False)
    wq = nc.declare_dram_parameter("wq", [QD, NPAIR, 104], BF16, isOutput=False)
    wq3p = nc.declare_dram_parameter("wq3p", [128, 2, 104], BF16, isOutput=False)
    wk = nc.declare_dram_parameter("wk", [CD, NPAIR, 104], BF16, isOutput=False)
    wv = nc.declare_dram_parameter("wv", [CD, QD], BF16, isOutput=False)
    wo = nc.declare_dram_parameter("wo", [NPAIR, 128, QD], BF16, isOutput=False)
    outT = nc.declare_dram_parameter("outT", [QD, NLOC], F32, isOutput=True)

    with tile.TileContext(nc) as tc:
        with (
            tc.tile_pool(name="consts", bufs=1) as consts,
            tc.tile_pool(name="xt", bufs=3) as xt_pool,
            tc.tile_pool(name="qt", bufs=2) as qt_pool,
            tc.tile_pool(name="ex", bufs=2) as ex_pool,
            tc.tile_pool(name="lz", bufs=3) as lz_pool,
            tc.tile_pool(name="rbr", bufs=3) as rbr_pool,
            tc.tile_pool(name="st", bufs=2) as st_pool,
            tc.tile_pool(name="oo", bufs=4) as oo_pool,
        ):
            # ---- load + stage constants ----
            def staged(shape, dtype, tag, src):
                s = consts.tile(shape, dtype, tag=f"s{tag}")
                nc.sync.dma_start(out=s, in_=src)
                t = consts.tile(shape, dtype, tag=tag)
                nc.vector.tensor_copy(out=t, in_=s)
                return t

            wq_sb = [
                staged([dk, NPAIR, 104], BF16, f"wq{i}", wq[d0 : d0 + dk, :, :])
                for i, (d0, dk) in enumerate(DK_Q[:2])
            ]
            # K=64 tail of the q contraction, pairs interleaved at partition
            # bases 0/64 so two pairs' tail matmuls row-tile concurrently
            wq3p_sb = staged([128, 2, 104], BF16, "wq3p", wq3p[:, :, :])
            wo_sb = [
                staged([128, QD], BF16, f"wo{p}", wo[p, :, :]) for p in range(NPAIR)
            ]
            wk_sb = [
                staged([dk, NPAIR, 104], BF16, f"wk{i}", wk[d0 : d0 + dk, :, :])
                for i, (d0, dk) in enumerate(DK_C)
            ]
            wv_sb = [
                staged([dk, QD], BF16, f"wv{i}", wv[d0 : d0 + dk, :])
                for i, (d0, dk) in enumerate(DK_C)
            ]
            ct_sb = [
                staged([dk, NKL], BF16, f"ct{i}", cT[d0 : d0 + dk, :])
                for i, (d0, dk) in enumerate(DK_C)
            ]
            # all-ones [77|64] stationary operand: rb = ones^T @ ex puts
            # Z = sum_k ex[k, n] in every psum row
            ones77 = consts.tile([NK, 64], BF16, tag="ones77")
            nc.vector.memset(ones77, 1.0)

            with (
                tc.tile_pool(name="ps_q", bufs=2, space="PSUM") as ps_q,
                tc.tile_pool(name="ps_sc", bufs=1, space="PSUM") as ps_sc,
                tc.tile_pool(name="ps_ot", bufs=2, space="PSUM") as ps_ot,
                tc.tile_pool(name="ps_rb", bufs=1, space="PSUM") as ps_rb,
                tc.tile_pool(name="ps_po", bufs=1, space="PSUM") as ps_po,
            ):
                # ---- setup projections ----
                kt_sb = []
                vp_sb = []
                # kT[p]: [104 | NKL], heads of pair p at partitions 0 / 64
                for p in range(NPAIR):
                    kp = ps_q.tile([104, NKL], F32, tag="q")
                    for i in range(len(DK_C)):
                        nc.tensor.matmul(
                            kp,
                            wk_sb[i][:, p, :],
                            ct_sb[i],
                            start=(i == 0),
                            stop=(i == len(DK_C) - 1),
                        )
                    t = consts.tile([104, NKL], BF16, tag=f"kt{p}")
                    nc.scalar.copy(out=t, in_=kp)
                    kt_sb.append(t)

                # vp[b]: [77 | 8*64]; head h: col 64h = 1 (Z), cols
                # 64h+1..64h+40 = v channels, rest 0
                for b in range(BL):
                    vb = ps_ot.tile([NK, QD], F32, tag="ot")
                    for i in range(len(DK_C)):
                        nc.tensor.matmul(
                            vb,
                            ct_sb[i][:, b * NK : (b + 1) * NK],
                            wv_sb[i],
                            start=(i == 0),
                            stop=(i == len(DK_C) - 1),
                        )
                    tf = consts.tile([NK, HEADS * 64], F32, tag=f"vpf{b}")
                    nc.vector.memset(tf, 0.0)
                    tf3 = tf.rearrange("p (h c) -> p h c", c=64)
                    vb3 = vb.rearrange("p (h c) -> p h c", c=DH)
                    nc.vector.memset(tf3[:, :, 0:1], 1.0)
                    nc.vector.tensor_copy(out=tf3[:, :, 1 : DH + 1], in_=vb3)
                    t = consts.tile([NK, HEADS * 64], BF16, tag=f"vp{b}")
                    nc.vector.tensor_copy(out=t, in_=tf)
                    vp_sb.append(t)

                # ---- software-pipelined main loop ----
                # round ci: q+scores(ci) | attention tail(ci-1) | out-proj(ci-2)
                exs_hist = {}   # ci -> dict p -> [ex_a, ex_b]
                sts_hist = {}   # ci -> list of st tiles per pair
                qts_cur = None

                def emit_q2(ci, g, xts, xt3d):
                    # pairs (2g, 2g+1): two K=128 chunks each, then the two
                    # K=64 tail matmuls run as concurrent row-tiles at array
                    # rows 0-63 / 64-127
                    pa, pb = 2 * g, 2 * g + 1
                    qpa = ps_q.tile([104, CHUNK], F32, tag="q")
                    for i in range(2):
                        nc.tensor.matmul(
                            qpa, wq_sb[i][:, pa, :], xts[i],
                            start=(i == 0), stop=False,
                        )
                    qpb = ps_q.tile([104, CHUNK], F32, tag="q")
                    for i in range(2):
                        nc.tensor.matmul(
                            qpb, wq_sb[i][:, pb, :], xts[i],
                            start=(i == 0), stop=False,
                        )
                    nc.tensor.matmul(
                        qpa, wq3p_sb[0:64, g, :], xt3d[0:64, :],
                        start=False, stop=True, skip_group_check=True,
                    )
                    nc.tensor.matmul(
                        qpb, wq3p_sb[64:128, g, :], xt3d[64:128, :],
                        start=False, stop=True, skip_group_check=True,
                    )
                    for p, qp in ((pa, qpa), (pb, qpb)):
                        qt = qt_pool.tile([104, CHUNK], BF16, tag=f"qt{p}")
                        nc.vector.tensor_copy(out=qt, in_=qp)
                        qts_cur[p] = qt

                def emit_sc(ci, p):
                    # both score halves of a pair in ONE [77|1024] psum tile
                    # (two adjacent banks): head A rows 0-39 (row tile (0,0))
                    # and head B rows 64-103 (tile (64,0)) share deps so the
                    # scheduler keeps them adjacent -> concurrent in the PE
                    # array; one exp covers both halves
                    b = ci // CHUNKS_PER_BATCH
                    bs = b * NK
                    sch = ps_sc.tile([NK, 2 * CHUNK], F32, tag="sc")
                    nc.tensor.matmul(
                        sch[:, 0:CHUNK],
                        kt_sb[p][0:DH, bs : bs + NK],
                        qts_cur[p][0:DH, :],
                        start=True,
                        stop=True,
                    )
                    nc.tensor.matmul(
                        sch[:, CHUNK : 2 * CHUNK],
                        kt_sb[p][64 : 64 + DH, bs : bs + NK],
                        qts_cur[p][64 : 64 + DH, :],
                        start=True,
                        stop=True,
                    )
                    exh = ex_pool.tile([NK, 2 * CHUNK], BF16, tag=f"ex{p}")
                    nc.scalar.activation(
                        out=exh, in_=sch, func=mybir.ActivationFunctionType.Exp
                    )
                    exs_hist[ci][p] = exh

                def emit_tail(ci, p):
                    # AV + Z matmuls (col-tiled); 1/Z = exp(-ln Z) on ACT
                    # (same table set as the softmax exp); normalize on DVE
                    # straight from the AV psum
                    b = ci // CHUNKS_PER_BATCH
                    exh = exs_hist[ci][p]
                    exa = exh[:, 0:CHUNK]
                    exb = exh[:, CHUNK : 2 * CHUNK]
                    ot = ps_ot.tile([128, CHUNK], F32, tag="ot")
                    nc.tensor.matmul(
                        ot[0:64, :],
                        vp_sb[b][:, (2 * p) * 64 : (2 * p) * 64 + 64],
                        exa,
                        start=True,
                        stop=True,
                    )
                    nc.tensor.matmul(
                        ot[64:128, :],
                        vp_sb[b][:, (2 * p + 1) * 64 : (2 * p + 1) * 64 + 64],
                        exb,
                        start=True,
                        stop=True,
                    )
                    rb = ps_rb.tile([128, CHUNK], F32, tag="rb")
                    nc.tensor.matmul(rb[0:64, :], ones77, exa, start=True, stop=True)
                    nc.tensor.matmul(
                        rb[64:128, :], ones77, exb, start=True, stop=True
                    )
                    lz = lz_pool.tile([128, CHUNK], F32, tag="lz")
                    nc.scalar.activation(
                        out=lz, in_=rb, func=mybir.ActivationFunctionType.Ln
                    )
                    rbr = rbr_pool.tile([128, CHUNK], F32, tag="rbr")
                    nc.scalar.activation(
                        out=rbr,
                        in_=lz,
                        func=mybir.ActivationFunctionType.Exp,
                        scale=-1.0,
                    )
                    st = st_pool.tile([128, CHUNK], BF16, tag=f"st{p}")
                    with nc.allow_low_precision(
                        reason="bf16 attention weights are within tolerance"
                    ):
                        nc.vector.tensor_mul(st, ot, rbr)
                    sts_hist[ci][p] = st

                def emit_po_j(ci, j):
                    j0, jw = JT[j]
                    po = ps_po.tile([128, CHUNK], F32, tag="po")
                    n0 = ci * CHUNK
                    if jw == 128:
                        for p in range(NPAIR):
                            nc.tensor.matmul(
                                po,
                                wo_sb[p][:, j0 : j0 + jw],
                                sts_hist[ci][p],
                                start=(p == 0),
                                stop=(p == NPAIR - 1),
                            )
                        oo = oo_pool.tile([jw, CHUNK], F32, tag="oo")
                        nc.vector.tensor_copy(out=oo, in_=po)
                    else:
                        # M=64 tile: pairs 0+1 accumulate into psum rows 0-63
                        # and pairs 2+3 into rows 64-127 (col-tiled,
                        # concurrent); the halves are summed during evac
                        for g in range(2):
                            nc.tensor.matmul(
                                po[64 * g : 64 * g + 64, :],
                                wo_sb[2 * g][:, j0 : j0 + jw],
                                sts_hist[ci][2 * g],
                                start=True,
                                stop=False,
                                skip_group_check=True,
                            )
                        for g in range(2):
                            nc.tensor.matmul(
                                po[64 * g : 64 * g + 64, :],
                                wo_sb[2 * g + 1][:, j0 : j0 + jw],
                                sts_hist[ci][2 * g + 1],
                                start=False,
                                stop=True,
                                skip_group_check=True,
                            )
                        oo = oo_pool.tile([jw, CHUNK], F32, tag="oo")
                        nc.vector.tensor_copy(out=oo, in_=po[0:64, :])
                        nc.vector.scalar_tensor_tensor(
                            out=oo,
                            in0=po[64:128, :],
                            scalar=1.0,
                            in1=oo,
                            op0=mybir.AluOpType.mult,
                            op1=mybir.AluOpType.add,
                        )
                    nc.sync.dma_start(
                        out=outT[j0 : j0 + jw, n0 : n0 + CHUNK], in_=oo
                    )

                for ci in range(NCHUNKS + 2):
                    cur = ci if ci < NCHUNKS else None
                    tl = ci - 1 if 0 <= ci - 1 < NCHUNKS else None
                    pp = ci - 2 if ci - 2 >= 0 else None

                    if cur is not None:
                        n0 = cur * CHUNK
                        xts = []
                        for i, (d0, dk) in enumerate(DK_Q[:2]):
                            t = xt_pool.tile([dk, CHUNK], BF16, tag=f"xt{i}")
                            nc.sync.dma_start(
                                out=t, in_=xT[d0 : d0 + dk, n0 : n0 + CHUNK]
                            )
                            xts.append(t)
                        # channels 256-319 replicated to partitions 0-63 AND
                        # 64-127 (replicate read from DRAM) for the row-tiled
                        # K=64 tail matmuls
                        xt3d = xt_pool.tile([128, CHUNK], BF16, tag="xt2")
                        x3 = xT[256:320, n0 : n0 + CHUNK]
                        nc.sync.dma_start(
                            out=xt3d,
                            in_=AP(
                                tensor=x3.tensor,
                                offset=x3.offset,
                                ap=[[0, 2], [NLOC, 64], [1, CHUNK]],
                            ),
                        )
                        exs_hist[cur] = {}
                        qts_cur = {}
                    if tl is not None:
                        sts_hist[tl] = [None] * NPAIR

                    # round-robin the ACT feed: tails contribute Ln/exp pairs,
                    # score halves contribute exps -- alternating keeps every
                    # psum pool's consumer close behind its producer
                    if cur is not None:
                        emit_q2(cur, 0, xts, xt3d)
                    if tl is not None:
                        emit_tail(tl, 0)
                    if cur is not None:
                        emit_sc(cur, 0)
                        emit_q2(cur, 1, xts, xt3d)
                    if tl is not None:
                        emit_tail(tl, 1)
                    if cur is not None:
                        emit_sc(cur, 1)
                    if tl is not None:
                        emit_tail(tl, 2)
                    if cur is not None:
                        emit_sc(cur, 2)
                    if tl is not None:
                        emit_tail(tl, 3)
                    if cur is not None:
                        emit_sc(cur, 3)
                    if pp is not None:
                        emit_po_j(pp, 0)
                        emit_po_j(pp, 1)
                        emit_po_j(pp, 2)
                        del sts_hist[pp]
                    if tl is not None:
                        del exs_hist[tl]

    return _split_multi_waits(nc)


_PROGRAM = None


def _get_program():
    global _PROGRAM
    if _PROGRAM is None:
        _PROGRAM = _build_program()
    return _PROGRAM


def _prep_weights(Wq, Wk, Wv, Wo, bo, gamma_q, gamma_k, gamma_v, gamma_out):
    scale = DH ** -0.5
    Wqp = (gamma_q[:, None] * Wq) * scale          # [320i, 320d]
    Wkp = gamma_k[:, None] * Wk                    # [320i, 768d]
    Wvp = gamma_v[:, None] * Wv                    # [320i, 768d]
    Wop = gamma_out[:, None] * Wo                  # [320j, 320i]
    bop = (gamma_out * bo).astype(np.float32)

    wq_dev = np.zeros((QD, NPAIR, 104), np.float32)
    wk_dev = np.zeros((CD, NPAIR, 104), np.float32)
    for p in range(NPAIR):
        hA, hB = 2 * p, 2 * p + 1
        wq_dev[:, p, 0:DH] = Wqp[hA * DH : (hA + 1) * DH, :].T
        wq_dev[:, p, 64 : 64 + DH] = Wqp[hB * DH : (hB + 1) * DH, :].T
        wk_dev[:, p, 0:DH] = Wkp[hA * DH : (hA + 1) * DH, :].T
        wk_dev[:, p, 64 : 64 + DH] = Wkp[hB * DH : (hB + 1) * DH, :].T
    wv_dev = np.ascontiguousarray(Wvp.T, dtype=np.float32)     # [768, 320]
    wq3p_dev = np.zeros((128, 2, 104), np.float32)
    for g in range(2):
        wq3p_dev[0:64, g, :] = wq_dev[256:320, 2 * g, :]
        wq3p_dev[64:128, g, :] = wq_dev[256:320, 2 * g + 1, :]
    # st rows per pair: 0 = Z/Z = 1 (bias rides here for pair 0), 1..40 =
    # head A channels, 64 = 1 (unused), 65..104 = head B channels, rest 0
    wo_dev = np.zeros((NPAIR, 128, QD), np.float32)
    for p in range(NPAIR):
        hA, hB = 2 * p, 2 * p + 1
        wo_dev[p, 1 : DH + 1, :] = Wop[:, hA * DH : (hA + 1) * DH].T
        wo_dev[p, 65 : 65 + DH, :] = Wop[:, hB * DH : (hB + 1) * DH].T
    wo_dev[0, 0, :] = bop
    return wq_dev, wq3p_dev, wk_dev, wv_dev, wo_dev


def kernel(x, context, Wq, Wk, Wv, Wo, bo, gamma_q, gamma_k, gamma_v, gamma_out):
    global LAST_EXEC_NS, LAST_RESULTS
    x = np.asarray(x, np.float32)
    context = np.asarray(context, np.float32)
    wq_dev, wq3p_dev, wk_dev, wv_dev, wo_dev = _prep_weights(
        np.asarray(Wq, np.float32), np.asarray(Wk, np.float32),
        np.asarray(Wv, np.float32), np.asarray(Wo, np.float32),
        np.asarray(bo, np.float32), np.asarray(gamma_q, np.float32),
        np.asarray(gamma_k, np.float32), np.asarray(gamma_v, np.float32),
        np.asarray(gamma_out, np.float32),
    )

    in_maps = []
    for c in range(NCORES):
        xs = x[c * BL : (c + 1) * BL].reshape(NLOC, QD)
        cs = context[c * BL : (c + 1) * BL].reshape(NKL, CD)
        in_maps.append(
            {
                "xT": np.ascontiguousarray(xs.T).astype(BF16NP),
                "cT": np.ascontiguousarray(cs.T).astype(BF16NP),
                "wq": wq_dev.astype(BF16NP),
                "wq3p": wq3p_dev.astype(BF16NP),
                "wk": wk_dev.astype(BF16NP),
                "wv": wv_dev.astype(BF16NP),
                "wo": wo_dev.astype(BF16NP),
            }
        )

    nc = _get_program()
    res = run_bass_kernel_spmd(nc, in_maps, list(range(NCORES)))
    LAST_EXEC_NS = res.exec_time_ns
    LAST_RESULTS = res

    out = np.empty((B, NQ, QD), np.float32)
    for c in range(NCORES):
        out[c * BL : (c + 1) * BL] = (
            np.asarray(res.results[c]["outT"]).T.reshape(BL, NQ, QD)
        )
    return out


# revision 22
# speedup vs baseline: 1.0137x; 1.0137x over previous
"""Cross-attention kernel for Trainium2 (8 NeuronCores, data-parallel over batch).

Reference computation (per batch b):
    q = (x @ Wq.T) * gamma_q ; k = (ctx @ Wk.T) * gamma_k ; v = (ctx @ Wv.T) * gamma_v
    per head: o = softmax(q k^T / sqrt(dh)) v
    out = (concat_heads(o) @ Wo.T + bo) * gamma_out

Device strategy (per core, 4 batches, n = 4*4096 = 16384 query rows):
  - Everything runs in "transposed world": activations live as [channel | n]
    so the contraction dim is always on partitions.
  - Host folds gamma_q/gamma_k/gamma_v/gamma_out and the 1/sqrt(dh) scale into
    the weights, pre-transposes x and context, and transposes the output back.
  - Heads are packed in PAIRS at partition bases {0, 64} (matmul operand base
    partitions must be 32-aligned and equal for lhsT/rhs).  Score matmuls of a
    pair are row-tiled (40-row K at array rows 0-63 / 64-127, concurrent);
    AV and Z matmuls are col-tiled (64-row M at psum rows 0-63 / 64-127,
    concurrent), landing directly in the [128|512] pair layout the output
    projection wants.
  - Softmax denominator: rb = ones[77|64]^T @ ex is a matmul whose every
    output row is Z = sum_k exp(s) for that column's query -- reduction and
    partition-broadcast in one PE op.  1/Z = exp(-ln Z) on the scalar engine
    (ln and exp live in the same activation table set), then st = ot * (1/Z)
    on the DVE straight from the AV psum.
  - V carries a leading ones-column, so ot row 0 (and 64) is Z; st row 0 is
    Z * (1/Z) = 1 and the output bias rides in wo[pair0, row0, :].
  - Software pipeline over chunks: round ci emits q+scores of chunk ci, the
    attention tail (AV/Z/normalize) of chunk ci-1, and the output projection
    of chunk ci-2, interleaved so the ACT exp chain and the DVE normalize
    chain never stall the PE.
"""

import os
import sys

import ml_dtypes
import numpy as np

BF16NP = ml_dtypes.bfloat16

for _p in ("/opt/trn_rl_repo",):
    if _p not in sys.path and os.path.isdir(_p):
        sys.path.append(_p)

import concourse.bass as bass
import concourse.mybir as mybir
import concourse.tile as tile
from concourse.bass import AP
from concourse.bass_utils import run_bass_kernel_spmd

HEADS = 8
DH = 40
QD = 320            # query/input channel dim == inner dim
CD = 768            # context channel dim
B, NQ, NK = 32, 4096, 77
NCORES = 8
BL = B // NCORES    # batches per core = 4
NLOC = BL * NQ      # query rows per core = 16384
NKL = BL * NK       # context rows per core = 308
CHUNK = 512
NCHUNKS = NLOC // CHUNK          # 32
CHUNKS_PER_BATCH = NQ // CHUNK   # 8
NPAIR = HEADS // 2               # 4 head pairs; pair p = heads (2p, 2p+1)

F32 = mybir.dt.float32
BF16 = mybir.dt.bfloat16

# K-chunking of the contraction dims
DK_Q = [(0, 128), (128, 128), (256, 64)]                       # QD = 320
DK_C = [(i * 128, 128) for i in range(6)]                      # CD = 768
JT = [(0, 128), (128, 128), (256, 64)]                         # out channels 320

LAST_EXEC_NS = None
LAST_RESULTS = None


def _split_multi_waits(nc):
    """Walrus codegen allows at most ONE semaphore wait per instruction.
    Split any instruction with N>1 waits into (N-1) same-engine NoOps, each
    carrying one wait, followed by the original instruction with the last
    wait. Engines execute their streams in order, so this is equivalent."""
    k = 0
    for blk in nc.m.functions[0].blocks:
        insts = list(blk.instructions)
        out = []
        for ins in insts:
            si = getattr(ins, "sync_info", None)
            if si is not None and len(si.on_wait) > 1:
                waits = list(si.on_wait)
                for w in waits[:-1]:
                    nop = mybir.InstNoOp(name=f"wsplit-{k}")
                    k += 1
                    nop.engine = ins.engine
                    nop.sync_info = mybir.SyncInfo(on_wait=[w], on_update=[])
                    out.append(nop)
                ins.sync_info = mybir.SyncInfo(
                    on_wait=[waits[-1]], on_update=list(si.on_update)
                )
            out.append(ins)
        if len(out) != len(insts):
            blk.instructions = out
    return nc


def _build_program():
    nc = bass.Bass(trn_type="TRN2")

    xT = nc.declare_dram_parameter("xT", [QD, NLOC], BF16, isOutput=False)
    cT = nc.declare_dram_parameter("cT", [CD, NKL], BF16, isOutput=False)
    wq = nc.declare_dram_parameter("wq", [QD, NPAIR, 104], BF16, isOutput=False)
    wq3p = nc.declare_dram_parameter("wq3p", [128, 2, 104], BF16, isOutput=False)
    wk = nc.declare_dram_parameter("wk", [CD, NPAIR, 104], BF16, isOutput=False)
    wv = nc.declare_dram_parameter("wv", [CD, QD], BF16, isOutput=False)
    wo = nc.declare_dram_parameter("wo", [NPAIR, 128, QD], BF16, isOutput=False)
    outT = nc.declare_dram_parameter("outT", [QD, NLOC], F32, isOutput=True)

    with tile.TileContext(nc) as tc:
        with (
            tc.tile_pool(name="consts", bufs=1) as consts,
            tc.tile_pool(name="xt", bufs=3) as xt_pool,
            tc.tile_pool(name="qt", bufs=2) as qt_pool,
            tc.tile_pool(name="ex", bufs=2) as ex_pool,
            tc.tile_pool(name="lz", bufs=3) as lz_pool,
            tc.tile_pool(name="rbr", bufs=3) as rbr_pool,
            tc.tile_pool(name="st", bufs=2) as st_pool,
            tc.tile_pool(name="oo", bufs=4) as oo_pool,
        ):
            # ---- load + stage constants ----
            def staged(shape, dtype, tag, src):
                s = consts.tile(shape, dtype, tag=f"s{tag}")
                nc.sync.dma_start(out=s, in_=src)
                t = consts.tile(shape, dtype, tag=tag)
                nc.vector.tensor_copy(out=t, in_=s)
                return t

            wq_sb = [
                staged([dk, NPAIR, 104], BF16, f"wq{i}", wq[d0 : d0 + dk, :, :])
                for i, (d0, dk) in enumerate(DK_Q[:2])
            ]
            # K=64 tail of the q contraction, pairs interleaved at partition
            # bases 0/64 so two pairs' tail matmuls row-tile concurrently
            wq3p_sb = staged([128, 2, 104], BF16, "wq3p", wq3p[:, :, :])
            wo_sb = [
                staged([128, QD], BF16, f"wo{p}", wo[p, :, :]) for p in range(NPAIR)
            ]
            wk_sb = [
                staged([dk, NPAIR, 104], BF16, f"wk{i}", wk[d0 : d0 + dk, :, :])
                for i, (d0, dk) in enumerate(DK_C)
            ]
            wv_sb = [
                staged([dk, QD], BF16, f"wv{i}", wv[d0 : d0 + dk, :])
                for i, (d0, dk) in enumerate(DK_C)
            ]
            ct_sb = [
                staged([dk, NKL], BF16, f"ct{i}", cT[d0 : d0 + dk, :])
                for i, (d0, dk) in enumerate(DK_C)
            ]
            # all-ones [77|64] stationary operand: rb = ones^T @ ex puts
            # Z = sum_k ex[k, n] in every psum row
            ones77 = consts.tile([NK, 64], BF16, tag="ones77")
            nc.vector.memset(ones77, 1.0)

            with (
                tc.tile_pool(name="ps_q", bufs=2, space="PSUM") as ps_q,
                tc.tile_pool(name="ps_sc", bufs=1, space="PSUM") as ps_sc,
                tc.tile_pool(name="ps_ot", bufs=2, space="PSUM") as ps_ot,
                tc.tile_pool(name="ps_rb", bufs=1, space="PSUM") as ps_rb,
                tc.tile_pool(name="ps_po", bufs=1, space="PSUM") as ps_po,
            ):
                # ---- setup projections ----
                kt_sb = []
                vp_sb = []
                # kT[p]: [104 | NKL], heads of pair p at partitions 0 / 64
                for p in range(NPAIR):
                    kp = ps_q.tile([104, NKL], F32, tag="q")
                    for i in range(len(DK_C)):
                        nc.tensor.matmul(
                            kp,
                            wk_sb[i][:, p, :],
                            ct_sb[i],
                            start=(i == 0),
                            stop=(i == len(DK_C) - 1),
                        )
                    t = consts.tile([104, NKL], BF16, tag=f"kt{p}")
                    nc.scalar.copy(out=t, in_=kp)
                    kt_sb.append(t)

                # vp[b]: [77 | 8*64]; head h: col 64h = 1 (Z), cols
                # 64h+1..64h+40 = v channels, rest 0
                for b in range(BL):
                    vb = ps_ot.tile([NK, QD], F32, tag="ot")
                    for i in range(len(DK_C)):
                        nc.tensor.matmul(
                            vb,
                            ct_sb[i][:, b * NK : (b + 1) * NK],
                            wv_sb[i],
                            start=(i == 0),
                            stop=(i == len(DK_C) - 1),
                        )
                    tf = consts.tile([NK, HEADS * 64], F32, tag=f"vpf{b}")
                    nc.vector.memset(tf, 0.0)
                    tf3 = tf.rearrange("p (h c) -> p h c", c=64)
                    vb3 = vb.rearrange("p (h c) -> p h c", c=DH)
                    nc.vector.memset(tf3[:, :, 0:1], 1.0)
                    nc.vector.tensor_copy(out=tf3[:, :, 1 : DH + 1], in_=vb3)
                    t = consts.tile([NK, HEADS * 64], BF16, tag=f"vp{b}")
                    nc.vector.tensor_copy(out=t, in_=tf)
                    vp_sb.append(t)

                # ---- software-pipelined main loop ----
                # round ci: q+scores(ci) | attention tail(ci-1) | out-proj(ci-2)
                exs_hist = {}   # ci -> dict p -> [ex_a, ex_b]
                sts_hist = {}   # ci -> list of st tiles per pair
                qts_cur = None

                def emit_q2(ci, g, xts, xt3d):
                    # pairs (2g, 2g+1): two K=128 chunks each, then the two
                    # K=64 tail matmuls run as concurrent row-tiles at array
                    # rows 0-63 / 64-127
                    pa, pb = 2 * g, 2 * g + 1
                    qpa = ps_q.tile([104, CHUNK], F32, tag="q")
                    for i in range(2):
                        nc.tensor.matmul(
                            qpa, wq_sb[i][:, pa, :], xts[i],
                            start=(i == 0), stop=False,
                        )
                    qpb = ps_q.tile([104, CHUNK], F32, tag="q")
                    for i in range(2):
                        nc.tensor.matmul(
                            qpb, wq_sb[i][:, pb, :], xts[i],
                            start=(i == 0), stop=False,
                        )
                    nc.tensor.matmul(
                        qpa, wq3p_sb[0:64, g, :], xt3d[0:64, :],
                        start=False, stop=True, skip_group_check=True,
                    )
                    nc.tensor.matmul(
                        qpb, wq3p_sb[64:128, g, :], xt3d[64:128, :],
                        start=False, stop=True, skip_group_check=True,
                    )
                    for p, qp in ((pa, qpa), (pb, qpb)):
                        qt = qt_pool.tile([104, CHUNK], BF16, tag=f"qt{p}")
                        nc.vector.tensor_copy(out=qt, in_=qp)
                        qts_cur[p] = qt

                def emit_sc(ci, p):
                    # both score halves of a pair in ONE [77|1024] psum tile
                    # (two adjacent banks): head A rows 0-39 (row tile (0,0))
                    # and head B rows 64-103 (tile (64,0)) share deps so the
                    # scheduler keeps them adjacent -> concurrent in the PE
                    # array; one exp covers both halves
                    b = ci // CHUNKS_PER_BATCH
                    bs = b * NK
                    sch = ps_sc.tile([NK, 2 * CHUNK], F32, tag="sc")
                    nc.tensor.matmul(
                        sch[:, 0:CHUNK],
                        kt_sb[p][0:DH, bs : bs + NK],
                        qts_cur[p][0:DH, :],
                        start=True,
                        stop=True,
                    )
                    nc.tensor.matmul(
                        sch[:, CHUNK : 2 * CHUNK],
                        kt_sb[p][64 : 64 + DH, bs : bs + NK],
                        qts_cur[p][64 : 64 + DH, :],
                        start=True,
                        stop=True,
                    )
                    exh = ex_pool.tile([NK, 2 * CHUNK], BF16, tag=f"ex{p}")
                    nc.scalar.activation(
                        out=exh, in_=sch, func=mybir.ActivationFunctionType.Exp
                    )
                    exs_hist[ci][p] = exh

                def emit_tail(ci, p):
                    # AV + Z matmuls (col-tiled); 1/Z = exp(-ln Z) on ACT
                    # (same table set as the softmax exp); normalize on DVE
                    # straight from the AV psum
                    b = ci // CHUNKS_PER_BATCH
                    exh = exs_hist[ci][p]
                    exa = exh[:, 0:CHUNK]
                    exb = exh[:, CHUNK : 2 * CHUNK]
                    ot = ps_ot.tile([128, CHUNK], F32, tag="ot")
                    nc.tensor.matmul(
                        ot[0:64, :],
                        vp_sb[b][:, (2 * p) * 64 : (2 * p) * 64 + 64],
                        exa,
                        start=True,
                        stop=True,
                    )
                    nc.tensor.matmul(
                        ot[64:128, :],
                        vp_sb[b][:, (2 * p + 1) * 64 : (2 * p + 1) * 64 + 64],
                        exb,
                        start=True,
                        stop=True,
                    )
                    rb = ps_rb.tile([128, CHUNK], F32, tag="rb")
                    nc.tensor.matmul(rb[0:64, :], ones77, exa, start=True, stop=True)
                    nc.tensor.matmul(
                        rb[64:128, :], ones77, exb, start=True, stop=True
                    )
                    lz = lz_pool.tile([128, CHUNK], F32, tag="lz")
                    nc.scalar.activation(
                        out=lz, in_=rb, func=mybir.ActivationFunctionType.Ln
                    )
                    rbr = rbr_pool.tile([128, CHUNK], F32, tag="rbr")
                    nc.scalar.activation(
                        out=rbr,
                        in_=lz,
                        func=mybir.ActivationFunctionType.Exp,
                        scale=-1.0,
                    )
                    st = st_pool.tile([128, CHUNK], BF16, tag=f"st{p}")
                    with nc.allow_low_precision(
                        reason="bf16 attention weights are within tolerance"
                    ):
                        nc.vector.tensor_mul(st, ot, rbr)
                    sts_hist[ci][p] = st

                def emit_po_j(ci, j):
                    j0, jw = JT[j]
                    po = ps_po.tile([128, CHUNK], F32, tag="po")
                    for p in range(NPAIR):
                        nc.tensor.matmul(
                            po[0:jw, :],
                            wo_sb[p][:, j0 : j0 + jw],
                            sts_hist[ci][p],
                            start=(p == 0),
                            stop=(p == NPAIR - 1),
                        )
                    oo = oo_pool.tile([jw, CHUNK], F32, tag="oo")
                    nc.vector.tensor_copy(out=oo, in_=po[0:jw, :])
                    n0 = ci * CHUNK
                    nc.sync.dma_start(
                        out=outT[j0 : j0 + jw, n0 : n0 + CHUNK], in_=oo
                    )

                for ci in range(NCHUNKS + 2):
                    cur = ci if ci < NCHUNKS else None
                    tl = ci - 1 if 0 <= ci - 1 < NCHUNKS else None
                    pp = ci - 2 if ci - 2 >= 0 else None

                    if cur is not None:
                        n0 = cur * CHUNK
                        xts = []
                        for i, (d0, dk) in enumerate(DK_Q[:2]):
                            t = xt_pool.tile([dk, CHUNK], BF16, tag=f"xt{i}")
                            nc.sync.dma_start(
                                out=t, in_=xT[d0 : d0 + dk, n0 : n0 + CHUNK]
                            )
                            xts.append(t)
                        # channels 256-319 replicated to partitions 0-63 AND
                        # 64-127 (replicate read from DRAM) for the row-tiled
                        # K=64 tail matmuls
                        xt3d = xt_pool.tile([128, CHUNK], BF16, tag="xt2")
                        x3 = xT[256:320, n0 : n0 + CHUNK]
                        nc.sync.dma_start(
                            out=xt3d,
                            in_=AP(
                                tensor=x3.tensor,
                                offset=x3.offset,
                                ap=[[0, 2], [NLOC, 64], [1, CHUNK]],
                            ),
                        )
                        exs_hist[cur] = {}
                        qts_cur = {}
                    if tl is not None:
                        sts_hist[tl] = [None] * NPAIR

                    # round-robin the ACT feed: tails contribute Ln/exp pairs,
                    # score halves contribute exps -- alternating keeps every
                    # psum pool's consumer close behind its producer
                    if cur is not None:
                        emit_q2(cur, 0, xts, xt3d)
                    if tl is not None:
                        emit_tail(tl, 0)
                    if cur is not None:
                        emit_sc(cur, 0)
                        emit_q2(cur, 1, xts, xt3d)
                    if tl is not None:
                        emit_tail(tl, 1)
                    if cur is not None:
                        emit_sc(cur, 1)
                    if tl is not None:
                        emit_tail(tl, 2)
                    if cur is not None:
                        emit_sc(cur, 2)
                    if tl is not None:
                        emit_tail(tl, 3)
                    if cur is not None:
                        emit_sc(cur, 3)
                    if pp is not None:
                        emit_po_j(pp, 0)
                        emit_po_j(pp, 1)
                        emit_po_j(pp, 2)
                        del sts_hist[pp]
                    if tl is not None:
                        del exs_hist[tl]

    return _split_multi_waits(nc)


_PROGRAM = None


def _get_program():
    global _PROGRAM
    if _PROGRAM is None:
        _PROGRAM = _build_program()
    return _PROGRAM


def _prep_weights(Wq, Wk, Wv, Wo, bo, gamma_q, gamma_k, gamma_v, gamma_out):
    scale = DH ** -0.5
    Wqp = (gamma_q[:, None] * Wq) * scale          # [320i, 320d]
    Wkp = gamma_k[:, None] * Wk                    # [320i, 768d]
    Wvp = gamma_v[:, None] * Wv                    # [320i, 768d]
    Wop = gamma_out[:, None] * Wo                  # [320j, 320i]
    bop = (gamma_out * bo).astype(np.float32)

    wq_dev = np.zeros((QD, NPAIR, 104), np.float32)
    wk_dev = np.zeros((CD, NPAIR, 104), np.float32)
    for p in range(NPAIR):
        hA, hB = 2 * p, 2 * p + 1
        wq_dev[:, p, 0:DH] = Wqp[hA * DH : (hA + 1) * DH, :].T
        wq_dev[:, p, 64 : 64 + DH] = Wqp[hB * DH : (hB + 1) * DH, :].T
        wk_dev[:, p, 0:DH] = Wkp[hA * DH : (hA + 1) * DH, :].T
        wk_dev[:, p, 64 : 64 + DH] = Wkp[hB * DH : (hB + 1) * DH, :].T
    wv_dev = np.ascontiguousarray(Wvp.T, dtype=np.float32)     # [768, 320]
    wq3p_dev = np.zeros((128, 2, 104), np.float32)
    for g in range(2):
        wq3p_dev[0:64, g, :] = wq_dev[256:320, 2 * g, :]
        wq3p_dev[64:128, g, :] = wq_dev[256:320, 2 * g + 1, :]
    # st rows per pair: 0 = Z/Z = 1 (bias rides here for pair 0), 1..40 =
    # head A channels, 64 = 1 (unused), 65..104 = head B channels, rest 0
    wo_dev = np.zeros((NPAIR, 128, QD), np.float32)
    for p in range(NPAIR):
        hA, hB = 2 * p, 2 * p + 1
        wo_dev[p, 1 : DH + 1, :] = Wop[:, hA * DH : (hA + 1) * DH].T
        wo_dev[p, 65 : 65 + DH, :] = Wop[:, hB * DH : (hB + 1) * DH].T
    wo_dev[0, 0, :] = bop
    return wq_dev, wq3p_dev, wk_dev, wv_dev, wo_dev


def kernel(x, context, Wq, Wk, Wv, Wo, bo, gamma_q, gamma_k, gamma_v, gamma_out):
    global LAST_EXEC_NS, LAST_RESULTS
    x = np.asarray(x, np.float32)
    context = np.asarray(context, np.float32)
    wq_dev, wq3p_dev, wk_dev, wv_dev, wo_dev = _prep_weights(
        np.asarray(Wq, np.float32), np.asarray(Wk, np.float32),
        np.asarray(Wv, np.float32), np.asarray(Wo, np.float32),
        np.asarray(bo, np.float32), np.asarray(gamma_q, np.float32),
        np.asarray(gamma_k, np.float32), np.asarray(gamma_v, np.float32),
        np.asarray(gamma_out, np.float32),
    )

    in_maps = []
    for c in range(NCORES):
        xs = x[c * BL : (c + 1) * BL].reshape(NLOC, QD)
        cs = context[c * BL : (c + 1) * BL].reshape(NKL, CD)
        in_maps.append(
            {
                "xT": np.ascontiguousarray(xs.T).astype(BF16NP),
                "cT": np.ascontiguousarray(cs.T).astype(BF16NP),
                "wq": wq_dev.astype(BF16NP),
                "wq3p": wq3p_dev.astype(BF16NP),
                "wk": wk_dev.astype(BF16NP),
                "wv": wv_dev.astype(BF16NP),
                "wo": wo_dev.astype(BF16NP),
            }
        )

    nc = _get_program()
    res = run_bass_kernel_spmd(nc, in_maps, list(range(NCORES)))
    LAST_EXEC_NS = res.exec_time_ns
    LAST_RESULTS = res

    out = np.empty((B, NQ, QD), np.float32)
    for c in range(NCORES):
        out[c * BL : (c + 1) * BL] = (
            np.asarray(res.results[c]["outT"]).T.reshape(BL, NQ, QD)
        )
    return out


# revision 23
# speedup vs baseline: 1.0204x; 1.0065x over previous
"""Cross-attention kernel for Trainium2 (8 NeuronCores, data-parallel over batch).

Reference computation (per batch b):
    q = (x @ Wq.T) * gamma_q ; k = (ctx @ Wk.T) * gamma_k ; v = (ctx @ Wv.T) * gamma_v
    per head: o = softmax(q k^T / sqrt(dh)) v
    out = (concat_heads(o) @ Wo.T + bo) * gamma_out

Device strategy (per core, 4 batches, n = 4*4096 = 16384 query rows):
  - Everything runs in "transposed world": activations live as [channel | n]
    so the contraction dim is always on partitions.
  - Host folds gamma_q/gamma_k/gamma_v/gamma_out and the 1/sqrt(dh) scale into
    the weights, pre-transposes x and context, and transposes the output back.
  - Heads are packed in PAIRS at partition bases {0, 64} (matmul operand base
    partitions must be 32-aligned and equal for lhsT/rhs).  Score matmuls of a
    pair are row-tiled (40-row K at array rows 0-63 / 64-127, concurrent);
    AV and Z matmuls are col-tiled (64-row M at psum rows 0-63 / 64-127,
    concurrent), landing directly in the [128|512] pair layout the output
    projection wants.
  - Softmax denominator: rb = ones[77|64]^T @ ex is a matmul whose every
    output row is Z = sum_k exp(s) for that column's query -- reduction and
    partition-broadcast in one PE op.  1/Z = exp(-ln Z) on the scalar engine
    (ln and exp live in the same activation table set), then st = ot * (1/Z)
    on the DVE straight from the AV psum.
  - V carries a leading ones-column, so ot row 0 (and 64) is Z; st row 0 is
    Z * (1/Z) = 1 and the output bias rides in wo[pair0, row0, :].
  - Software pipeline over chunks: round ci emits q+scores of chunk ci, the
    attention tail (AV/Z/normalize) of chunk ci-1, and the output projection
    of chunk ci-2, interleaved so the ACT exp chain and the DVE normalize
    chain never stall the PE.
"""

import os
import sys

import ml_dtypes
import numpy as np

BF16NP = ml_dtypes.bfloat16

for _p in ("/opt/trn_rl_repo",):
    if _p not in sys.path and os.path.isdir(_p):
        sys.path.append(_p)

import concourse.bass as bass
import concourse.mybir as mybir
import concourse.tile as tile
from concourse.bass import AP
from concourse.bass_utils import run_bass_kernel_spmd

HEADS = 8
DH = 40
QD = 320            # query/input channel dim == inner dim
CD = 768            # context channel dim
B, NQ, NK = 32, 4096, 77
NCORES = 8
BL = B // NCORES    # batches per core = 4
NLOC = BL * NQ      # query rows per core = 16384
NKL = BL * NK       # context rows per core = 308
CHUNK = 512
NCHUNKS = NLOC // CHUNK          # 32
CHUNKS_PER_BATCH = NQ // CHUNK   # 8
NPAIR = HEADS // 2               # 4 head pairs; pair p = heads (2p, 2p+1)

F32 = mybir.dt.float32
BF16 = mybir.dt.bfloat16

# K-chunking of the contraction dims
DK_Q = [(0, 128), (128, 128), (256, 64)]                       # QD = 320
DK_C = [(i * 128, 128) for i in range(6)]                      # CD = 768
JT = [(0, 128), (128, 128), (256, 64)]                         # out channels 320

LAST_EXEC_NS = None
LAST_RESULTS = None


def _split_multi_waits(nc):
    """Walrus codegen allows at most ONE semaphore wait per instruction.
    Split any instruction with N>1 waits into (N-1) same-engine NoOps, each
    carrying one wait, followed by the original instruction with the last
    wait. Engines execute their streams in order, so this is equivalent."""
    k = 0
    for blk in nc.m.functions[0].blocks:
        insts = list(blk.instructions)
        out = []
        for ins in insts:
            si = getattr(ins, "sync_info", None)
            if si is not None and len(si.on_wait) > 1:
                waits = list(si.on_wait)
                for w in waits[:-1]:
                    nop = mybir.InstNoOp(name=f"wsplit-{k}")
                    k += 1
                    nop.engine = ins.engine
                    nop.sync_info = mybir.SyncInfo(on_wait=[w], on_update=[])
                    out.append(nop)
                ins.sync_info = mybir.SyncInfo(
                    on_wait=[waits[-1]], on_update=list(si.on_update)
                )
            out.append(ins)
        if len(out) != len(insts):
            blk.instructions = out
    return nc


def _build_program():
    nc = bass.Bass(trn_type="TRN2")

    xT = nc.declare_dram_parameter("xT", [QD, NLOC], BF16, isOutput=False)
    cT = nc.declare_dram_parameter("cT", [CD, NKL], BF16, isOutput=False)
    wq = nc.declare_dram_parameter("wq", [QD, NPAIR, 104], BF16, isOutput=False)
    wq3p = nc.declare_dram_parameter("wq3p", [128, 2, 104], BF16, isOutput=False)
    wk = nc.declare_dram_parameter("wk", [CD, NPAIR, 104], BF16, isOutput=False)
    wv = nc.declare_dram_parameter("wv", [CD, QD], BF16, isOutput=False)
    wo = nc.declare_dram_parameter("wo", [NPAIR, 128, QD], BF16, isOutput=False)
    outT = nc.declare_dram_parameter("outT", [QD, NLOC], F32, isOutput=True)

    with tile.TileContext(nc) as tc:
        with (
            tc.tile_pool(name="consts", bufs=1) as consts,
            tc.tile_pool(name="xt", bufs=3) as xt_pool,
            tc.tile_pool(name="qt", bufs=2) as qt_pool,
            tc.tile_pool(name="ex", bufs=2) as ex_pool,
            tc.tile_pool(name="lz", bufs=3) as lz_pool,
            tc.tile_pool(name="rbr", bufs=3) as rbr_pool,
            tc.tile_pool(name="st", bufs=2) as st_pool,
            tc.tile_pool(name="oo", bufs=4) as oo_pool,
        ):
            # ---- load + stage constants ----
            def staged(shape, dtype, tag, src):
                # DMA straight into the persistent tile -- DMA-landed SBUF
                # tiles are valid matmul operands, no staging copy needed
                t = consts.tile(shape, dtype, tag=tag)
                nc.sync.dma_start(out=t, in_=src)
                return t

            wq_sb = [
                staged([dk, NPAIR, 104], BF16, f"wq{i}", wq[d0 : d0 + dk, :, :])
                for i, (d0, dk) in enumerate(DK_Q[:2])
            ]
            # K=64 tail of the q contraction, pairs interleaved at partition
            # bases 0/64 so two pairs' tail matmuls row-tile concurrently
            wq3p_sb = staged([128, 2, 104], BF16, "wq3p", wq3p[:, :, :])
            wo_sb = [
                staged([128, QD], BF16, f"wo{p}", wo[p, :, :]) for p in range(NPAIR)
            ]
            wk_sb = [
                staged([dk, NPAIR, 104], BF16, f"wk{i}", wk[d0 : d0 + dk, :, :])
                for i, (d0, dk) in enumerate(DK_C)
            ]
            wv_sb = [
                staged([dk, QD], BF16, f"wv{i}", wv[d0 : d0 + dk, :])
                for i, (d0, dk) in enumerate(DK_C)
            ]
            ct_sb = [
                staged([dk, NKL], BF16, f"ct{i}", cT[d0 : d0 + dk, :])
                for i, (d0, dk) in enumerate(DK_C)
            ]
            # all-ones [77|64] stationary operand: rb = ones^T @ ex puts
            # Z = sum_k ex[k, n] in every psum row
            ones77 = consts.tile([NK, 64], BF16, tag="ones77")
            nc.vector.memset(ones77, 1.0)

            with (
                tc.tile_pool(name="ps_q", bufs=2, space="PSUM") as ps_q,
                tc.tile_pool(name="ps_sc", bufs=1, space="PSUM") as ps_sc,
                tc.tile_pool(name="ps_ot", bufs=2, space="PSUM") as ps_ot,
                tc.tile_pool(name="ps_rb", bufs=1, space="PSUM") as ps_rb,
                tc.tile_pool(name="ps_po", bufs=1, space="PSUM") as ps_po,
            ):
                # ---- setup projections ----
                kt_sb = []
                vp_sb = []
                # kT[p]: [104 | NKL], heads of pair p at partitions 0 / 64
                for p in range(NPAIR):
                    kp = ps_q.tile([104, NKL], F32, tag="q")
                    for i in range(len(DK_C)):
                        nc.tensor.matmul(
                            kp,
                            wk_sb[i][:, p, :],
                            ct_sb[i],
                            start=(i == 0),
                            stop=(i == len(DK_C) - 1),
                        )
                    t = consts.tile([104, NKL], BF16, tag=f"kt{p}")
                    nc.scalar.copy(out=t, in_=kp)
                    kt_sb.append(t)

                # vp[b]: [77 | 8*64]; head h: col 64h = 1 (Z), cols
                # 64h+1..64h+40 = v channels, rest 0
                for b in range(BL):
                    vb = ps_ot.tile([NK, QD], F32, tag="ot")
                    for i in range(len(DK_C)):
                        nc.tensor.matmul(
                            vb,
                            ct_sb[i][:, b * NK : (b + 1) * NK],
                            wv_sb[i],
                            start=(i == 0),
                            stop=(i == len(DK_C) - 1),
                        )
                    tf = consts.tile([NK, HEADS * 64], F32, tag=f"vpf{b}")
                    nc.vector.memset(tf, 0.0)
                    tf3 = tf.rearrange("p (h c) -> p h c", c=64)
                    vb3 = vb.rearrange("p (h c) -> p h c", c=DH)
                    nc.vector.memset(tf3[:, :, 0:1], 1.0)
                    nc.vector.tensor_copy(out=tf3[:, :, 1 : DH + 1], in_=vb3)
                    t = consts.tile([NK, HEADS * 64], BF16, tag=f"vp{b}")
                    nc.vector.tensor_copy(out=t, in_=tf)
                    vp_sb.append(t)

                # ---- software-pipelined main loop ----
                # round ci: q+scores(ci) | attention tail(ci-1) | out-proj(ci-2)
                exs_hist = {}   # ci -> dict p -> [ex_a, ex_b]
                sts_hist = {}   # ci -> list of st tiles per pair
                qts_cur = None

                def emit_q2(ci, g, xts, xt3d):
                    # pairs (2g, 2g+1): two K=128 chunks each, then the two
                    # K=64 tail matmuls run as concurrent row-tiles at array
                    # rows 0-63 / 64-127
                    pa, pb = 2 * g, 2 * g + 1
                    qpa = ps_q.tile([104, CHUNK], F32, tag="q")
                    for i in range(2):
                        nc.tensor.matmul(
                            qpa, wq_sb[i][:, pa, :], xts[i],
                            start=(i == 0), stop=False,
                        )
                    qpb = ps_q.tile([104, CHUNK], F32, tag="q")
                    for i in range(2):
                        nc.tensor.matmul(
                            qpb, wq_sb[i][:, pb, :], xts[i],
                            start=(i == 0), stop=False,
                        )
                    nc.tensor.matmul(
                        qpa, wq3p_sb[0:64, g, :], xt3d[0:64, :],
                        start=False, stop=True, skip_group_check=True,
                    )
                    nc.tensor.matmul(
                        qpb, wq3p_sb[64:128, g, :], xt3d[64:128, :],
                        start=False, stop=True, skip_group_check=True,
                    )
                    for p, qp in ((pa, qpa), (pb, qpb)):
                        qt = qt_pool.tile([104, CHUNK], BF16, tag=f"qt{p}")
                        nc.vector.tensor_copy(out=qt, in_=qp)
                        qts_cur[p] = qt

                def emit_sc(ci, p):
                    # both score halves of a pair in ONE [77|1024] psum tile
                    # (two adjacent banks): head A rows 0-39 (row tile (0,0))
                    # and head B rows 64-103 (tile (64,0)) share deps so the
                    # scheduler keeps them adjacent -> concurrent in the PE
                    # array; one exp covers both halves
                    b = ci // CHUNKS_PER_BATCH
                    bs = b * NK
                    sch = ps_sc.tile([NK, 2 * CHUNK], F32, tag="sc")
                    nc.tensor.matmul(
                        sch[:, 0:CHUNK],
                        kt_sb[p][0:DH, bs : bs + NK],
                        qts_cur[p][0:DH, :],
                        start=True,
                        stop=True,
                    )
                    nc.tensor.matmul(
                        sch[:, CHUNK : 2 * CHUNK],
                        kt_sb[p][64 : 64 + DH, bs : bs + NK],
                        qts_cur[p][64 : 64 + DH, :],
                        start=True,
                        stop=True,
                    )
                    exh = ex_pool.tile([NK, 2 * CHUNK], BF16, tag=f"ex{p}")
                    nc.scalar.activation(
                        out=exh, in_=sch, func=mybir.ActivationFunctionType.Exp
                    )
                    exs_hist[ci][p] = exh

                def emit_tail(ci, p):
                    # AV + Z matmuls (col-tiled); 1/Z = exp(-ln Z) on ACT
                    # (same table set as the softmax exp); normalize on DVE
                    # straight from the AV psum
                    b = ci // CHUNKS_PER_BATCH
                    exh = exs_hist[ci][p]
                    exa = exh[:, 0:CHUNK]
                    exb = exh[:, CHUNK : 2 * CHUNK]
                    ot = ps_ot.tile([128, CHUNK], F32, tag="ot")
                    nc.tensor.matmul(
                        ot[0:64, :],
                        vp_sb[b][:, (2 * p) * 64 : (2 * p) * 64 + 64],
                        exa,
                        start=True,
                        stop=True,
                    )
                    nc.tensor.matmul(
                        ot[64:128, :],
                        vp_sb[b][:, (2 * p + 1) * 64 : (2 * p + 1) * 64 + 64],
                        exb,
                        start=True,
                        stop=True,
                    )
                    rb = ps_rb.tile([128, CHUNK], F32, tag="rb")
                    nc.tensor.matmul(rb[0:64, :], ones77, exa, start=True, stop=True)
                    nc.tensor.matmul(
                        rb[64:128, :], ones77, exb, start=True, stop=True
                    )
                    lz = lz_pool.tile([128, CHUNK], F32, tag="lz")
                    nc.scalar.activation(
                        out=lz, in_=rb, func=mybir.ActivationFunctionType.Ln
                    )
                    rbr = rbr_pool.tile([128, CHUNK], F32, tag="rbr")
                    nc.scalar.activation(
                        out=rbr,
                        in_=lz,
                        func=mybir.ActivationFunctionType.Exp,
                        scale=-1.0,
                    )
                    st = st_pool.tile([128, CHUNK], BF16, tag=f"st{p}")
                    with nc.allow_low_precision(
                        reason="bf16 attention weights are within tolerance"
                    ):
                        nc.vector.tensor_mul(st, ot, rbr)
                    sts_hist[ci][p] = st

                def emit_po_j(ci, j):
                    j0, jw = JT[j]
                    po = ps_po.tile([128, CHUNK], F32, tag="po")
                    for p in range(NPAIR):
                        nc.tensor.matmul(
                            po[0:jw, :],
                            wo_sb[p][:, j0 : j0 + jw],
                            sts_hist[ci][p],
                            start=(p == 0),
                            stop=(p == NPAIR - 1),
                        )
                    oo = oo_pool.tile([jw, CHUNK], F32, tag="oo")
                    nc.vector.tensor_copy(out=oo, in_=po[0:jw, :])
                    n0 = ci * CHUNK
                    nc.sync.dma_start(
                        out=outT[j0 : j0 + jw, n0 : n0 + CHUNK], in_=oo
                    )

                for ci in range(NCHUNKS + 2):
                    cur = ci if ci < NCHUNKS else None
                    tl = ci - 1 if 0 <= ci - 1 < NCHUNKS else None
                    pp = ci - 2 if ci - 2 >= 0 else None

                    if cur is not None:
                        n0 = cur * CHUNK
                        xts = []
                        for i, (d0, dk) in enumerate(DK_Q[:2]):
                            t = xt_pool.tile([dk, CHUNK], BF16, tag=f"xt{i}")
                            nc.sync.dma_start(
                                out=t, in_=xT[d0 : d0 + dk, n0 : n0 + CHUNK]
                            )
                            xts.append(t)
                        # channels 256-319 replicated to partitions 0-63 AND
                        # 64-127 (replicate read from DRAM) for the row-tiled
                        # K=64 tail matmuls
                        xt3d = xt_pool.tile([128, CHUNK], BF16, tag="xt2")
                        x3 = xT[256:320, n0 : n0 + CHUNK]
                        nc.sync.dma_start(
                            out=xt3d,
                            in_=AP(
                                tensor=x3.tensor,
                                offset=x3.offset,
                                ap=[[0, 2], [NLOC, 64], [1, CHUNK]],
                            ),
                        )
                        exs_hist[cur] = {}
                        qts_cur = {}
                    if tl is not None:
                        sts_hist[tl] = [None] * NPAIR

                    # round-robin the ACT feed: tails contribute Ln/exp pairs,
                    # score halves contribute exps -- alternating keeps every
                    # psum pool's consumer close behind its producer
                    if cur is not None:
                        emit_q2(cur, 0, xts, xt3d)
                    if tl is not None:
                        emit_tail(tl, 0)
                    if cur is not None:
                        emit_sc(cur, 0)
                        emit_q2(cur, 1, xts, xt3d)
                    if tl is not None:
                        emit_tail(tl, 1)
                    if cur is not None:
                        emit_sc(cur, 1)
                    if tl is not None:
                        emit_tail(tl, 2)
                    if cur is not None:
                        emit_sc(cur, 2)
                    if tl is not None:
                        emit_tail(tl, 3)
                    if cur is not None:
                        emit_sc(cur, 3)
                    if pp is not None:
                        emit_po_j(pp, 0)
                        emit_po_j(pp, 1)
                        emit_po_j(pp, 2)
                        del sts_hist[pp]
                    if tl is not None:
                        del exs_hist[tl]

    return _split_multi_waits(nc)


_PROGRAM = None


def _get_program():
    global _PROGRAM
    if _PROGRAM is None:
        _PROGRAM = _build_program()
    return _PROGRAM


def _prep_weights(Wq, Wk, Wv, Wo, bo, gamma_q, gamma_k, gamma_v, gamma_out):
    scale = DH ** -0.5
    Wqp = (gamma_q[:, None] * Wq) * scale          # [320i, 320d]
    Wkp = gamma_k[:, None] * Wk                    # [320i, 768d]
    Wvp = gamma_v[:, None] * Wv                    # [320i, 768d]
    Wop = gamma_out[:, None] * Wo                  # [320j, 320i]
    bop = (gamma_out * bo).astype(np.float32)

    wq_dev = np.zeros((QD, NPAIR, 104), np.float32)
    wk_dev = np.zeros((CD, NPAIR, 104), np.float32)
    for p in range(NPAIR):
        hA, hB = 2 * p, 2 * p + 1
        wq_dev[:, p, 0:DH] = Wqp[hA * DH : (hA + 1) * DH, :].T
        wq_dev[:, p, 64 : 64 + DH] = Wqp[hB * DH : (hB + 1) * DH, :].T
        wk_dev[:, p, 0:DH] = Wkp[hA * DH : (hA + 1) * DH, :].T
        wk_dev[:, p, 64 : 64 + DH] = Wkp[hB * DH : (hB + 1) * DH, :].T
    wv_dev = np.ascontiguousarray(Wvp.T, dtype=np.float32)     # [768, 320]
    wq3p_dev = np.zeros((128, 2, 104), np.float32)
    for g in range(2):
        wq3p_dev[0:64, g, :] = wq_dev[256:320, 2 * g, :]
        wq3p_dev[64:128, g, :] = wq_dev[256:320, 2 * g + 1, :]
    # st rows per pair: 0 = Z/Z = 1 (bias rides here for pair 0), 1..40 =
    # head A channels, 64 = 1 (unused), 65..104 = head B channels, rest 0
    wo_dev = np.zeros((NPAIR, 128, QD), np.float32)
    for p in range(NPAIR):
        hA, hB = 2 * p, 2 * p + 1
        wo_dev[p, 1 : DH + 1, :] = Wop[:, hA * DH : (hA + 1) * DH].T
        wo_dev[p, 65 : 65 + DH, :] = Wop[:, hB * DH : (hB + 1) * DH].T
    wo_dev[0, 0, :] = bop
    return wq_dev, wq3p_dev, wk_dev, wv_dev, wo_dev


def kernel(x, context, Wq, Wk, Wv, Wo, bo, gamma_q, gamma_k, gamma_v, gamma_out):
    global LAST_EXEC_NS, LAST_RESULTS
    x = np.asarray(x, np.float32)
    context = np.asarray(context, np.float32)
    wq_dev, wq3p_dev, wk_dev, wv_dev, wo_dev = _prep_weights(
        np.asarray(Wq, np.float32), np.asarray(Wk, np.float32),
        np.asarray(Wv, np.float32), np.asarray(Wo, np.float32),
        np.asarray(bo, np.float32), np.asarray(gamma_q, np.float32),
        np.asarray(gamma_k, np.float32), np.asarray(gamma_v, np.float32),
        np.asarray(gamma_out, np.float32),
    )

    in_maps = []
    for c in range(NCORES):
        xs = x[c * BL : (c + 1) * BL].reshape(NLOC, QD)
        cs = context[c * BL : (c + 1) * BL].reshape(NKL, CD)
        in_maps.append(
            {
                "xT": np.ascontiguousarray(xs.T).astype(BF16NP),
                "cT": np.ascontiguousarray(cs.T).astype(BF16NP),
                "wq": wq_dev.astype(BF16NP),
                "wq3p": wq3p_dev.astype(BF16NP),
                "wk": wk_dev.astype(BF16NP),
                "wv": wv_dev.astype(BF16NP),
                "wo": wo_dev.astype(BF16NP),
            }
        )

    nc = _get_program()
    res = run_bass_kernel_spmd(nc, in_maps, list(range(NCORES)))
    LAST_EXEC_NS = res.exec_time_ns
    LAST_RESULTS = res

    out = np.empty((B, NQ, QD), np.float32)
    for c in range(NCORES):
        out[c * BL : (c + 1) * BL] = (
            np.asarray(res.results[c]["outT"]).T.reshape(BL, NQ, QD)
        )
    return out
